# revision 27
# baseline (speedup 1.0000x reference)
"""Multi-head attention (B=2, C=64, H=W=64, nh=8) on 8 TRN2 NeuronCores.

Sharding: 16 (batch, head) pairs -> 2 consecutive heads per core.
core = 4*b + p handles batch b, heads {2p, 2p+1} = conv channels [16p, 16p+16).

Dispatch architecture. The wall clock here is dominated by the axon tunnel
(~50ms base per call, ~45MB/s each way), so the kernel minimizes bytes on
the wire, not FLOPs:
  - x is uploaded ONCE as int8 (0.5MB), quantized per row (= per input
    channel) on the host; the dequant scales are folded into the conv
    weights host-side, so the device never sees them.
  - the upload is concentrated into two 256KB single-device messages (small
    per-shard messages measured ~1.8x slower per byte): x[b] goes to the
    group leader (core 0 / core 4) and a bass-internal AllReduce over
    replica groups [[0..3],[4..7]] against persistent zero shards
    broadcasts it to the batch group on-chip.
  - conv weights ship as fp16 (70KB).
  - the device returns gamma*attn (NOT the residual sum) as int8 with
    per-(row, 512-block) scales riding as bitcast bytes in the last 32
    columns of the same tensor (one fetch, no tiny-message overhead); the
    host adds the exact f32 residual x.
  - zero output operands live on device permanently (no donation, never
    re-uploaded).
Quantization error (numpy sim): ~1.1e-3 rel, vs the 2e-2 gate.

Per-core bass pipeline (all on-chip, scores never hit HBM):
  int8 x -> f32 -> fp32r; conv1x1 (PE fp32r, bias via ones-row)
  -> DRAM-bounce gathers to build Q^T/K^T [8,4096] (fp32r) and V-chunked
     [128,32,33] (bf16, ones column for the softmax denominator)
     (torch .view semantics: Q[n,d] = conv[ch0 + n//512, (n%512)*8 + d])
  -> S^T tiles [128m, 512n] = K Q^T on PE (fp32r), exp on ACT with
     1/sqrt(8) folded into the activation scale, bf16 output
     (no max subtraction; scores are O(1) by construction)
  -> PV via augmented V|ones bf16 matmul -> [33, 512] psum (row 32 = denom)
     (gamma is folded into wv/bv host-side, so pv is already gamma-scaled)
  -> normalize: broadcast denom via PE ones outer product, DVE reciprocal,
     pv * recip (tensor_mul)
  -> absmax-quantize each [8,512] tile to int8, emit scales.
"""

import numpy as np

import concourse.bacc as bacc
import concourse.bass as bass
import concourse.tile as tile
from concourse import mybir

F32 = mybir.dt.float32
F32R = mybir.dt.float32r
BF16 = mybir.dt.bfloat16
F16 = mybir.dt.float16
I8 = mybir.dt.int8

B = 2
C = 64
N = 4096          # H*W
NH = 8
HD = 8            # head dim
HPC = 2           # heads per core
CH = HPC * HD     # 16 conv channels per core
NCORES = 8
NBLK = N // 512   # 8 n-blocks of 512 query positions
MCHUNK = N // 128  # 32 m-chunks of 128 key positions
EXPW = 1536       # elements exp'd per ACT instruction (psum banks = EXPW/512)
ST_BUFS = 2       # st tile double buffering
ACC_BUFS = 2      # PV accumulator buffering
SCALE = 1.0 / np.sqrt(float(HD))


def _chunk_groups():
    """Partition the 32 m-chunks into groups of <= EXPW//512 for one exp each."""
    per = EXPW // 512
    groups, k = [], 0
    while k < MCHUNK:
        n = min(per, MCHUNK - k)
        groups.append(list(range(k, k + n)))
        k += n
    return groups


def _emit(tc, xs_d, wcat_d, out_d, scr):
    nc = tc.nc

    with (
        tc.tile_pool(name="persist", bufs=1) as per,
        tc.tile_pool(name="ptp", bufs=3) as ptp,
        tc.tile_pool(name="epl", bufs=3) as epl,
        tc.tile_pool(name="stp", bufs=ST_BUFS, space="PSUM") as stp,
        tc.tile_pool(name="accp", bufs=ACC_BUFS, space="PSUM") as accp,
        tc.tile_pool(name="dram", bufs=1, space="DRAM") as dram,
    ):
        # ---- broadcast the full int8 x[b] from core 0 (resp. 4) ----
        # Host uploads x[b] as ONE 256KB message to the group leader; the
        # other cores' xs inputs are persistent device-resident zeros, so
        # AllReduce(add) within the group is a broadcast. (Collectives can't
        # touch I/O tensors -> bounce through internal DRAM.)
        xs_bounce = dram.tile([C, N], I8)
        xfull = dram.tile([C, N], I8)
        nc.gpsimd.dma_start(out=xs_bounce[:], in_=xs_d[:])
        nc.gpsimd.collective_compute(
            "AllReduce",
            mybir.AluOpType.add,
            replica_groups=[[0, 1, 2, 3], [4, 5, 6, 7]],
            ins=[xs_bounce.opt()],
            outs=[xfull.opt()],
        )

        # ---- persistent tiles ----
        ones8r = per.tile([1, HD], F32R)          # broadcast lhsT
        scl = [per.tile([HD, NBLK], F32, name=f"scl{h}", tag=f"scl{h}")
               for h in range(HPC)]
        qt = [per.tile([HD, N], F32R, name=f"qt{h}", tag=f"qt{h}") for h in range(HPC)]
        kt = [per.tile([HD, N], F32R, name=f"kt{h}", tag=f"kt{h}") for h in range(HPC)]
        vc = [per.tile([128, MCHUNK, 33], BF16, name=f"vc{h}", tag=f"vc{h}")
              for h in range(HPC)]

        o8f = per.tile([1, HD], F32)
        nc.vector.memset(o8f, 1.0)
        nc.vector.tensor_copy(ones8r, o8f)        # rounds to fp32r

        with tc.tile_pool(name="convin", bufs=1) as cin:
            xq8 = cin.tile([C, N], I8)            # gathered int8 x[b]
            nc.sync.dma_start(out=xq8[:], in_=xfull[:])
            xba = cin.tile([C + 1, N], F32)       # x[b] + ones row (bias)
            xbar = cin.tile([C + 1, N], F32R)
            nc.vector.memset(xba[C : C + 1, :], 1.0)
            nc.vector.tensor_copy(xba[0:C, :], xq8)   # int8 -> f32 (exact)
            nc.vector.tensor_copy(xbar, xba)      # rounds to fp32r

            wcf16 = cin.tile([C + 1, 3 * CH], F16)
            nc.sync.dma_start(out=wcf16[:], in_=wcat_d[:])
            wcf = cin.tile([C + 1, 3 * CH], F32)
            nc.vector.tensor_copy(wcf, wcf16)     # f16 -> f32 (exact)
            wcr = cin.tile([C + 1, 3 * CH], F32R)
            nc.vector.tensor_copy(wcr, wcf)       # rounds to fp32r

            # ---- conv1x1: [16,4096] = wT.T @ [65,4096] per q/k/v ----
            with tc.tile_pool(name="convout", bufs=1) as cop:
                cqkv = [
                    cop.tile([CH, N], F32R if t < 2 else BF16, name=f"c{t}", tag=f"c{t}")
                    for t in range(3)
                ]
                for t in range(3):
                    lhsT = wcr[:, t * CH : (t + 1) * CH]
                    for j in range(NBLK):
                        ps = stp.tile([128, EXPW], F32, tag="st")
                        nc.tensor.matmul(
                            ps[0:CH, 0:512],
                            lhsT=lhsT,
                            rhs=xbar[:, j * 512 : (j + 1) * 512],
                            start=True,
                            stop=True,
                        )
                        nc.vector.tensor_copy(
                            cqkv[t][:, j * 512 : (j + 1) * 512], ps[0:CH, 0:512]
                        )
                    nc.sync.dma_start(out=scr[t][:], in_=cqkv[t][:])


                # ---- re-layout gathers (torch .view semantics) ----
                for h in range(HPC):
                    rows = slice(h * HD, (h + 1) * HD)
                    # Q^T/K^T [d, n]: element = conv[row r, col 8t+d], n = 512r+t
                    for r0 in range(0, HD, 2):
                        nc.sync.dma_start(
                            out=qt[h][:].rearrange("d (r t) -> d r t", r=HD)[
                                :, r0 : r0 + 2, :
                            ],
                            in_=scr[0][rows, :].rearrange("r (t d) -> d r t", d=HD)[
                                :, r0 : r0 + 2, :
                            ],
                        )
                        nc.sync.dma_start(
                            out=kt[h][:].rearrange("d (r t) -> d r t", r=HD)[
                                :, r0 : r0 + 2, :
                            ],
                            in_=scr[1][rows, :].rearrange("r (t d) -> d r t", d=HD)[
                                :, r0 : r0 + 2, :
                            ],
                        )
                    # V chunked [i, chunk, d]: m = 128*chunk + i, chunk = 4r+tb
                    nc.sync.dma_start(
                        out=vc[h][:, :, 0:HD],
                        in_=scr[2][rows, :].rearrange(
                            "r (tb i d) -> i (r tb) d", tb=4, i=128, d=HD
                        ),
                    )
                    nc.vector.memset(vc[h][:, :, HD:32], 0.0)
                    nc.vector.memset(vc[h][:, :, 32:33], 1.0)

        # ---- attention per head / n-block ----
        for h in range(HPC):
            och = slice(h * HD, (h + 1) * HD)
            for j in range(NBLK):
                qblk = qt[h][:, j * 512 : (j + 1) * 512]
                acc = accp.tile([33, 512], F32, tag="acc")
                for grp in _chunk_groups():
                    st = stp.tile([128, EXPW], F32, tag="st")
                    for u, k in enumerate(grp):
                        nc.tensor.matmul(
                            st[:, u * 512 : (u + 1) * 512],
                            lhsT=kt[h][:, k * 128 : (k + 1) * 128],
                            rhs=qblk,
                            start=True,
                            stop=True,
                        )
                    w = len(grp) * 512
                    pt = ptp.tile([128, EXPW], BF16)
                    nc.scalar.activation(
                        pt[:, 0:w], st[:, 0:w],
                        mybir.ActivationFunctionType.Exp, scale=SCALE
                    )
                    for u, k in enumerate(grp):
                        nc.tensor.matmul(
                            acc[:, :],
                            lhsT=vc[h][:, k, :],
                            rhs=pt[:, u * 512 : (u + 1) * 512],
                            start=(k == 0),
                            stop=(k == MCHUNK - 1),
                        )

                # ---- epilogue: q8 = int8(gamma * pv / denom), per-row scale ----
                sb = epl.tile([1, 512], F32R, tag="sb")
                nc.vector.tensor_copy(sb, acc[32:33, :])  # denom -> fp32r
                rb = accp.tile([33, 512], F32, tag="acc")
                nc.tensor.matmul(
                    rb[0:HD, :], lhsT=ones8r, rhs=sb, start=True, stop=True
                )
                rbs = epl.tile([HD, 512], F32, tag="rbs")
                nc.vector.reciprocal(rbs, rb[0:HD, :])
                # gamma is folded into wv/bv host-side, so pv is already
                # gamma * attn * denom
                prod = epl.tile([HD, 512], F32, tag="prod")
                nc.vector.tensor_mul(prod, acc[0:HD, :], rbs)
                # per-row absmax -> scale = max(|row|,eps)/127, inv = 1/scale
                rmax = epl.tile([HD, 1], F32, tag="rmax")
                nc.vector.tensor_reduce(
                    rmax, prod, axis=mybir.AxisListType.XYZW,
                    op=mybir.AluOpType.max, apply_absolute_value=True,
                )
                sct = epl.tile([HD, 1], F32, tag="sct")
                nc.vector.tensor_scalar(
                    sct, rmax, 1e-30, 1.0 / 127.0,
                    op0=mybir.AluOpType.max, op1=mybir.AluOpType.mult,
                )
                nc.vector.tensor_copy(scl[h][:, j : j + 1], sct)
                inv = epl.tile([HD, 1], F32, tag="inv")
                nc.vector.reciprocal(inv, sct)
                qi8 = epl.tile([HD, 512], I8, tag="qi8")
                nc.vector.tensor_scalar_mul(qi8, prod, inv[:])
                nc.sync.dma_start(
                    out=out_d[och, j * 512 : (j + 1) * 512], in_=qi8
                )
        # scales ride in the last 32 byte-columns of out (f32 bytes, bitcast)
        for h in range(HPC):
            nc.sync.dma_start(
                out=out_d[h * HD : (h + 1) * HD, N : N + 4 * NBLK],
                in_=scl[h][:].bitcast(I8),
            )


def build_bass():
    nc = bacc.Bacc("TRN2", target_bir_lowering=False, debug=False, num_devices=NCORES)
    xs_d = nc.dram_tensor("xs", [C, N], I8, kind="ExternalInput").ap()
    wcat_d = nc.dram_tensor("wcat", [C + 1, 3 * CH], F16, kind="ExternalInput").ap()
    out_d = nc.dram_tensor("out", [CH, N + 4 * NBLK], I8, kind="ExternalOutput").ap()
    scr = [
        nc.dram_tensor("scr0", [CH, N], F32R).ap(),
        nc.dram_tensor("scr1", [CH, N], F32R).ap(),
        nc.dram_tensor("scr2", [CH, N], BF16).ap(),
    ]

    with tile.TileContext(nc) as tc:
        _emit(tc, xs_d, wcat_d, out_d, scr)
    nc.finalize()
    return nc


# ---------------- host / dispatch side ----------------

_STATE = None


def _build_state():
    import jax
    from jax.sharding import Mesh, PartitionSpec, NamedSharding
    from jax.experimental.shard_map import shard_map
    from concourse import bass2jax

    nc = build_bass()
    bass2jax.install_neuronx_cc_hook()

    partition_name = nc.partition_id_tensor.name if nc.partition_id_tensor else None
    in_names, out_names, out_avals = [], [], []
    for alloc in nc.m.functions[0].allocations:
        if not isinstance(alloc, mybir.MemoryLocationSet):
            continue
        name = alloc.memorylocations[0].name
        if alloc.kind == "ExternalInput":
            if name != partition_name:
                in_names.append(name)
        elif alloc.kind == "ExternalOutput":
            out_names.append(name)
            out_avals.append(
                jax.core.ShapedArray(tuple(alloc.tensor_shape), mybir.dt.np(alloc.dtype))
            )
    n_params = len(in_names)
    all_names = list(in_names) + out_names
    bind_names = list(all_names)
    if partition_name is not None:
        bind_names.append(partition_name)

    devices = jax.devices()[:NCORES]
    assert len(devices) == NCORES, f"need {NCORES} devices, got {len(jax.devices())}"
    mesh = Mesh(np.asarray(devices), ("core",))
    P = PartitionSpec
    csh = NamedSharding(mesh, P("core"))

    # --- bass exec jit (no donation: zero out-operands live on device forever) ---
    def _bass_body(*args):
        operands = list(args)
        if partition_name is not None:
            operands.append(bass2jax.partition_id_tensor())
        outs = bass2jax._bass_exec_p.bind(
            *operands,
            out_avals=tuple(out_avals),
            in_names=tuple(bind_names),
            out_names=tuple(out_names),
            lowering_input_output_aliases=(),
            sim_require_finite=True,
            sim_require_nnan=True,
            nc=nc,
        )
        return tuple(outs)

    run = jax.jit(
        shard_map(_bass_body, mesh=mesh,
                  in_specs=(P("core"),) * (n_params + len(out_names)),
                  out_specs=(P("core"),) * len(out_names),
                  check_rep=False),
        keep_unused=True,
    )

    zeros = [
        jax.device_put(
            np.zeros((NCORES * av.shape[0], *av.shape[1:]), av.dtype), csh
        )
        for av in out_avals
    ]

    # persistent zero xs shards for the non-leader cores (AllReduce broadcast)
    z64 = np.zeros((C, N), np.int8)
    xs_zero_shards = [
        None if c in (0, 4) else jax.device_put(z64, devices[c])
        for c in range(NCORES)
    ]

    return {
        "run": run,
        "zeros": zeros,
        "in_names": in_names,
        "out_names": out_names,
        "csh": csh,
        "jax": jax,
        "devices": devices,
        "xs_zero_shards": xs_zero_shards,
    }


def _get_state():
    global _STATE
    if _STATE is None:
        _STATE = _build_state()
    return _STATE


def _host_pre(x, wq, bq, wk, bk, wv, bv, gamma):
    """Quantize x per row to int8; fold the scales into fp16 conv weights."""
    xg = np.asarray(x, np.float32).reshape(B * C, N)
    if not xg.flags.c_contiguous:
        xg = np.ascontiguousarray(xg)
    s = np.abs(xg).max(axis=1) / 127.0            # (128,) per-channel scale
    np.maximum(s, 1e-30, out=s)
    xq = np.rint(xg * (1.0 / s)[:, None]).astype(np.int8)

    g = float(np.asarray(gamma, np.float32).reshape(-1)[0])
    wcat = np.empty((NCORES, C + 1, 3 * CH), np.float32)
    # gamma folded into wv/bv: gamma*(p@v)/d == (p@(gamma v))/d
    ws = {0: np.asarray(wq, np.float32), 1: np.asarray(wk, np.float32),
          2: np.asarray(wv, np.float32) * g}
    bs = {0: np.asarray(bq, np.float32), 1: np.asarray(bk, np.float32),
          2: np.asarray(bv, np.float32) * g}
    for b in range(B):
        sb = s[b * C : (b + 1) * C]               # this batch's channel scales
        for t in range(3):
            wt = ws[t] * sb[None, :]              # fold x scales into weights
            for p in range(4):
                core = 4 * b + p
                sl = slice(CH * p, CH * (p + 1))
                wcat[core, :C, t * CH : (t + 1) * CH] = wt[sl].T
                wcat[core, C, t * CH : (t + 1) * CH] = bs[t][sl]
    return xg, xq, wcat.reshape(NCORES * (C + 1), 3 * CH).astype(np.float16)


def kernel(x, wq, bq, wk, bk, wv, bv, gamma):
    st = _get_state()
    jax = st["jax"]
    xg, xq, wcat_all = _host_pre(x, wq, bq, wk, bk, wv, bv, gamma)
    # x[b] goes to the group leader (core 0 / core 4) as one 256KB message;
    # the other cores get the persistent zero shards (no transfer).
    shards = list(st["xs_zero_shards"])
    shards[0] = jax.device_put(xq[:C], st["devices"][0])
    shards[4] = jax.device_put(xq[C:], st["devices"][4])
    xs_g = jax.make_array_from_single_device_arrays(
        (NCORES * C, N), st["csh"], shards
    )
    arrs = {"xs": xs_g, "wcat": wcat_all}
    outs = st["run"](*[arrs[n] for n in st["in_names"]], *st["zeros"])
    aug = jax.device_get(outs[0])                 # (128, 4096+32) int8
    i8 = aug[:, :N]
    sc = np.ascontiguousarray(aug[:, N:]).view(np.float32)   # (128, 8)
    ga = i8.reshape(B * C, NBLK, 512) * sc.reshape(B * C, NBLK, 1)
    out = ga.reshape(B * C, N)
    out += xg                                     # exact f32 residual
    return out.reshape(B, C, 64, 64)


if __name__ == "__main__":
    rng = np.random.default_rng(0)
    x = rng.standard_normal((B, C, 64, 64), dtype=np.float32)
    wq, wk, wv = (
        rng.standard_normal((C, C), dtype=np.float32) / 8.0 for _ in range(3)
    )
    bq, bk, bv = (
        rng.standard_normal((C,), dtype=np.float32) * 0.01 for _ in range(3)
    )
    gamma = rng.random((1,), dtype=np.float32)
    out = kernel(x, wq, bq, wk, bk, wv, bv, gamma)
    print(out.shape, out.dtype)


# revision 28
# speedup vs baseline: 1.0561x; 1.0561x over previous
"""Multi-head attention (B=2, C=64, H=W=64, nh=8) on 8 TRN2 NeuronCores.

Sharding: 16 (batch, head) pairs -> 2 consecutive heads per core.
core = 4*b + p handles batch b, heads {2p, 2p+1} = conv channels [16p, 16p+16).

Dispatch architecture. The wall clock here is dominated by the axon tunnel
(~50ms base per call, ~45MB/s each way), so the kernel minimizes bytes on
the wire, not FLOPs:
  - x is uploaded ONCE as int8 (0.5MB), quantized per row (= per input
    channel) on the host; the dequant scales are folded into the conv
    weights host-side, so the device never sees them.
  - the upload is concentrated into two 256KB single-device messages (small
    per-shard messages measured ~1.8x slower per byte): x[b] goes to the
    group leader (core 0 / core 4) and a bass-internal AllReduce over
    replica groups [[0..3],[4..7]] against persistent zero shards
    broadcasts it to the batch group on-chip.
  - conv weights ship as fp16 (70KB).
  - the device returns gamma*attn (NOT the residual sum) as int8 with
    per-(row, 512-block) scales riding as bitcast bytes in the last 32
    columns of the same tensor (one fetch, no tiny-message overhead); the
    host adds the exact f32 residual x.
  - zero output operands live on device permanently (no donation, never
    re-uploaded).
Quantization error (numpy sim): ~1.1e-3 rel, vs the 2e-2 gate.

Per-core bass pipeline (all on-chip, scores never hit HBM):
  int8 x -> f32 -> fp32r; conv1x1 (PE fp32r, bias via ones-row)
  -> DRAM-bounce gathers to build Q^T/K^T [8,4096] (fp32r) and V-chunked
     [128,32,33] (bf16, ones column for the softmax denominator)
     (torch .view semantics: Q[n,d] = conv[ch0 + n//512, (n%512)*8 + d])
  -> S^T tiles [128m, 512n] = K Q^T on PE (fp32r), exp on ACT with
     1/sqrt(8) folded into the activation scale, bf16 output
     (no max subtraction; scores are O(1) by construction)
  -> PV via augmented V|ones bf16 matmul -> [33, 512] psum (row 32 = denom)
     (gamma is folded into wv/bv host-side, so pv is already gamma-scaled)
  -> normalize: broadcast denom via PE ones outer product, DVE reciprocal,
     pv * recip (tensor_mul)
  -> absmax-quantize each [8,512] tile to int8, emit scales.
"""

import numpy as np

import concourse.bacc as bacc
import concourse.bass as bass
import concourse.tile as tile
from concourse import mybir

F32 = mybir.dt.float32
F32R = mybir.dt.float32r
BF16 = mybir.dt.bfloat16
F16 = mybir.dt.float16
I8 = mybir.dt.int8

B = 2
C = 64
N = 4096          # H*W
NH = 8
HD = 8            # head dim
HPC = 2           # heads per core
CH = HPC * HD     # 16 conv channels per core
NCORES = 8
NBLK = N // 512   # 8 n-blocks of 512 query positions
MCHUNK = N // 128  # 32 m-chunks of 128 key positions
EXPW = 1536       # elements exp'd per ACT instruction (psum banks = EXPW/512)
ST_BUFS = 2       # st tile double buffering
ACC_BUFS = 2      # PV accumulator buffering
SCALE = 1.0 / np.sqrt(float(HD))


def _chunk_groups():
    """Partition the 32 m-chunks into groups of <= EXPW//512 for one exp each."""
    per = EXPW // 512
    groups, k = [], 0
    while k < MCHUNK:
        n = min(per, MCHUNK - k)
        groups.append(list(range(k, k + n)))
        k += n
    return groups


def _emit(tc, xs_d, wcat_d, out_d, scr):
    nc = tc.nc

    with (
        tc.tile_pool(name="persist", bufs=1) as per,
        tc.tile_pool(name="ptp", bufs=3) as ptp,
        tc.tile_pool(name="epl", bufs=3) as epl,
        tc.tile_pool(name="stp", bufs=ST_BUFS, space="PSUM") as stp,
        tc.tile_pool(name="accp", bufs=ACC_BUFS, space="PSUM") as accp,
        tc.tile_pool(name="dram", bufs=1, space="DRAM") as dram,
    ):
        # ---- broadcast the full int8 x[b] from core 0 (resp. 4) ----
        # Host uploads x[b] as ONE 256KB message to the group leader; the
        # other cores' xs inputs are persistent device-resident zeros, so
        # AllReduce(add) within the group is a broadcast. (Collectives can't
        # touch I/O tensors -> bounce through internal DRAM.)
        xs_bounce = dram.tile([C, N], I8)
        xfull = dram.tile([C, N], I8)
        nc.gpsimd.dma_start(out=xs_bounce[:], in_=xs_d[:])
        nc.gpsimd.collective_compute(
            "AllReduce",
            mybir.AluOpType.add,
            replica_groups=[[0, 1, 2, 3], [4, 5, 6, 7]],
            ins=[xs_bounce.opt()],
            outs=[xfull.opt()],
        )

        # ---- persistent tiles ----
        ones8r = per.tile([1, HD], F32R)          # broadcast lhsT
        scl = [per.tile([HD, NBLK], F32, name=f"scl{h}", tag=f"scl{h}")
               for h in range(HPC)]
        qt = [per.tile([HD, N], F32R, name=f"qt{h}", tag=f"qt{h}") for h in range(HPC)]
        kt = [per.tile([HD, N], F32R, name=f"kt{h}", tag=f"kt{h}") for h in range(HPC)]
        vc = [per.tile([128, MCHUNK, 33], BF16, name=f"vc{h}", tag=f"vc{h}")
              for h in range(HPC)]

        o8f = per.tile([1, HD], F32)
        nc.vector.memset(o8f, 1.0)
        nc.vector.tensor_copy(ones8r, o8f)        # rounds to fp32r

        with tc.tile_pool(name="convin", bufs=1) as cin:
            xq8 = cin.tile([C, N], I8)            # gathered int8 x[b]
            nc.sync.dma_start(out=xq8[:], in_=xfull[:])
            xba = cin.tile([C + 1, N], F32)       # x[b] + ones row (bias)
            xbar = cin.tile([C + 1, N], F32R)
            nc.vector.memset(xba[C : C + 1, :], 1.0)
            nc.vector.tensor_copy(xba[0:C, :], xq8)   # int8 -> f32 (exact)
            nc.vector.tensor_copy(xbar, xba)      # rounds to fp32r

            wcf16 = cin.tile([C + 1, 3 * CH], F16)
            nc.sync.dma_start(out=wcf16[:], in_=wcat_d[:])
            wcf = cin.tile([C + 1, 3 * CH], F32)
            nc.vector.tensor_copy(wcf, wcf16)     # f16 -> f32 (exact)
            wcr = cin.tile([C + 1, 3 * CH], F32R)
            nc.vector.tensor_copy(wcr, wcf)       # rounds to fp32r

            # ---- conv1x1: [16,4096] = wT.T @ [65,4096] per q/k/v ----
            with tc.tile_pool(name="convout", bufs=1) as cop:
                cqkv = [
                    cop.tile([CH, N], F32R if t < 2 else BF16, name=f"c{t}", tag=f"c{t}")
                    for t in range(3)
                ]
                for t in range(3):
                    lhsT = wcr[:, t * CH : (t + 1) * CH]
                    for j in range(NBLK):
                        ps = stp.tile([128, EXPW], F32, tag="st")
                        nc.tensor.matmul(
                            ps[0:CH, 0:512],
                            lhsT=lhsT,
                            rhs=xbar[:, j * 512 : (j + 1) * 512],
                            start=True,
                            stop=True,
                        )
                        nc.vector.tensor_copy(
                            cqkv[t][:, j * 512 : (j + 1) * 512], ps[0:CH, 0:512]
                        )
                    nc.sync.dma_start(out=scr[t][:], in_=cqkv[t][:])


                # ---- re-layout gathers (torch .view semantics) ----
                for h in range(HPC):
                    rows = slice(h * HD, (h + 1) * HD)
                    # Q^T/K^T [d, n]: element = conv[row r, col 8t+d], n = 512r+t
                    for r0 in range(0, HD, 2):
                        nc.sync.dma_start(
                            out=qt[h][:].rearrange("d (r t) -> d r t", r=HD)[
                                :, r0 : r0 + 2, :
                            ],
                            in_=scr[0][rows, :].rearrange("r (t d) -> d r t", d=HD)[
                                :, r0 : r0 + 2, :
                            ],
                        )
                        nc.sync.dma_start(
                            out=kt[h][:].rearrange("d (r t) -> d r t", r=HD)[
                                :, r0 : r0 + 2, :
                            ],
                            in_=scr[1][rows, :].rearrange("r (t d) -> d r t", d=HD)[
                                :, r0 : r0 + 2, :
                            ],
                        )
                    # V chunked [i, chunk, d]: m = 128*chunk + i, chunk = 4r+tb
                    nc.sync.dma_start(
                        out=vc[h][:, :, 0:HD],
                        in_=scr[2][rows, :].rearrange(
                            "r (tb i d) -> i (r tb) d", tb=4, i=128, d=HD
                        ),
                    )
                    nc.vector.memset(vc[h][:, :, HD:32], 0.0)
                    nc.vector.memset(vc[h][:, :, 32:33], 1.0)

        # ---- attention per head / n-block ----
        for h in range(HPC):
            och = slice(h * HD, (h + 1) * HD)
            for j in range(NBLK):
                qblk = qt[h][:, j * 512 : (j + 1) * 512]
                acc = accp.tile([33, 512], F32, tag="acc")
                for grp in _chunk_groups():
                    st = stp.tile([128, EXPW], F32, tag="st")
                    for u, k in enumerate(grp):
                        nc.tensor.matmul(
                            st[:, u * 512 : (u + 1) * 512],
                            lhsT=kt[h][:, k * 128 : (k + 1) * 128],
                            rhs=qblk,
                            start=True,
                            stop=True,
                        )
                    w = len(grp) * 512
                    pt = ptp.tile([128, EXPW], BF16)
                    nc.scalar.activation(
                        pt[:, 0:w], st[:, 0:w],
                        mybir.ActivationFunctionType.Exp, scale=SCALE
                    )
                    for u, k in enumerate(grp):
                        nc.tensor.matmul(
                            acc[:, :],
                            lhsT=vc[h][:, k, :],
                            rhs=pt[:, u * 512 : (u + 1) * 512],
                            start=(k == 0),
                            stop=(k == MCHUNK - 1),
                        )

                # ---- epilogue: q8 = int8(gamma * pv / denom), per-row scale ----
                sb = epl.tile([1, 512], F32R, tag="sb")
                nc.vector.tensor_copy(sb, acc[32:33, :])  # denom -> fp32r
                rb = accp.tile([33, 512], F32, tag="acc")
                nc.tensor.matmul(
                    rb[0:HD, :], lhsT=ones8r, rhs=sb, start=True, stop=True
                )
                rbs = epl.tile([HD, 512], F32, tag="rbs")
                nc.vector.reciprocal(rbs, rb[0:HD, :])
                # gamma is folded into wv/bv host-side, so pv is already
                # gamma * attn * denom
                prod = epl.tile([HD, 512], F32, tag="prod")
                nc.vector.tensor_mul(prod, acc[0:HD, :], rbs)
                # per-row absmax -> scale = max(|row|,eps)/127, inv = 1/scale
                rmax = epl.tile([HD, 1], F32, tag="rmax")
                nc.vector.tensor_reduce(
                    rmax, prod, axis=mybir.AxisListType.XYZW,
                    op=mybir.AluOpType.max, apply_absolute_value=True,
                )
                sct = epl.tile([HD, 1], F32, tag="sct")
                nc.vector.tensor_scalar(
                    sct, rmax, 1e-30, 1.0 / 127.0,
                    op0=mybir.AluOpType.max, op1=mybir.AluOpType.mult,
                )
                nc.vector.tensor_copy(scl[h][:, j : j + 1], sct)
                inv = epl.tile([HD, 1], F32, tag="inv")
                nc.vector.reciprocal(inv, sct)
                qi8 = epl.tile([HD, 512], I8, tag="qi8")
                nc.vector.tensor_scalar_mul(qi8, prod, inv[:])
                nc.sync.dma_start(
                    out=out_d[och, j * 512 : (j + 1) * 512], in_=qi8
                )
        # scales ride in the last 32 byte-columns of out (f32 bytes, bitcast)
        for h in range(HPC):
            nc.sync.dma_start(
                out=out_d[h * HD : (h + 1) * HD, N : N + 4 * NBLK],
                in_=scl[h][:].bitcast(I8),
            )


def build_bass():
    nc = bacc.Bacc("TRN2", target_bir_lowering=False, debug=False, num_devices=NCORES)
    xs_d = nc.dram_tensor("xs", [C, N], I8, kind="ExternalInput").ap()
    wcat_d = nc.dram_tensor("wcat", [C + 1, 3 * CH], F16, kind="ExternalInput").ap()
    out_d = nc.dram_tensor("out", [CH, N + 4 * NBLK], I8, kind="ExternalOutput").ap()
    scr = [
        nc.dram_tensor("scr0", [CH, N], F32R).ap(),
        nc.dram_tensor("scr1", [CH, N], F32R).ap(),
        nc.dram_tensor("scr2", [CH, N], BF16).ap(),
    ]

    with tile.TileContext(nc) as tc:
        _emit(tc, xs_d, wcat_d, out_d, scr)
    nc.finalize()
    return nc


# ---------------- host / dispatch side ----------------

_STATE = None


def _build_state():
    import jax
    from jax.sharding import Mesh, PartitionSpec, NamedSharding
    from jax.experimental.shard_map import shard_map
    from concourse import bass2jax

    nc = build_bass()
    bass2jax.install_neuronx_cc_hook()

    partition_name = nc.partition_id_tensor.name if nc.partition_id_tensor else None
    in_names, out_names, out_avals = [], [], []
    for alloc in nc.m.functions[0].allocations:
        if not isinstance(alloc, mybir.MemoryLocationSet):
            continue
        name = alloc.memorylocations[0].name
        if alloc.kind == "ExternalInput":
            if name != partition_name:
                in_names.append(name)
        elif alloc.kind == "ExternalOutput":
            out_names.append(name)
            out_avals.append(
                jax.core.ShapedArray(tuple(alloc.tensor_shape), mybir.dt.np(alloc.dtype))
            )
    n_params = len(in_names)
    all_names = list(in_names) + out_names
    bind_names = list(all_names)
    if partition_name is not None:
        bind_names.append(partition_name)

    devices = jax.devices()[:NCORES]
    assert len(devices) == NCORES, f"need {NCORES} devices, got {len(jax.devices())}"
    mesh = Mesh(np.asarray(devices), ("core",))
    P = PartitionSpec
    csh = NamedSharding(mesh, P("core"))

    # --- bass exec jit (no donation: zero out-operands live on device forever) ---
    def _bass_body(*args):
        operands = list(args)
        if partition_name is not None:
            operands.append(bass2jax.partition_id_tensor())
        outs = bass2jax._bass_exec_p.bind(
            *operands,
            out_avals=tuple(out_avals),
            in_names=tuple(bind_names),
            out_names=tuple(out_names),
            lowering_input_output_aliases=(),
            sim_require_finite=True,
            sim_require_nnan=True,
            nc=nc,
        )
        return tuple(outs)

    run = jax.jit(
        shard_map(_bass_body, mesh=mesh,
                  in_specs=(P("core"),) * (n_params + len(out_names)),
                  out_specs=(P("core"),) * len(out_names),
                  check_rep=False),
        keep_unused=True,
    )

    zeros = [
        jax.device_put(
            np.zeros((NCORES * av.shape[0], *av.shape[1:]), av.dtype), csh
        )
        for av in out_avals
    ]

    # persistent zero xs shards for the non-leader cores (AllReduce broadcast)
    z64 = np.zeros((C, N), np.int8)
    xs_zero_shards = [
        None if c in (0, 4) else jax.device_put(z64, devices[c])
        for c in range(NCORES)
    ]

    return {
        "run": run,
        "zeros": zeros,
        "in_names": in_names,
        "out_names": out_names,
        "csh": csh,
        "jax": jax,
        "devices": devices,
        "xs_zero_shards": xs_zero_shards,
    }


def _get_state():
    global _STATE
    if _STATE is None:
        _STATE = _build_state()
    return _STATE


def _host_pre(x, wq, bq, wk, bk, wv, bv, gamma):
    """Quantize x per row to int8; fold the scales into fp16 conv weights."""
    xg = np.asarray(x, np.float32).reshape(B * C, N)
    if not xg.flags.c_contiguous:
        xg = np.ascontiguousarray(xg)
    s = np.abs(xg).max(axis=1) / 127.0            # (128,) per-channel scale
    np.maximum(s, 1e-30, out=s)
    xq = np.rint(xg * (1.0 / s)[:, None]).astype(np.int8)

    g = float(np.asarray(gamma, np.float32).reshape(-1)[0])
    wcat = np.empty((NCORES, C + 1, 3 * CH), np.float32)
    # gamma folded into wv/bv: gamma*(p@v)/d == (p@(gamma v))/d
    ws = {0: np.asarray(wq, np.float32), 1: np.asarray(wk, np.float32),
          2: np.asarray(wv, np.float32) * g}
    bs = {0: np.asarray(bq, np.float32), 1: np.asarray(bk, np.float32),
          2: np.asarray(bv, np.float32) * g}
    for b in range(B):
        sb = s[b * C : (b + 1) * C]               # this batch's channel scales
        for t in range(3):
            wt = ws[t] * sb[None, :]              # fold x scales into weights
            for p in range(4):
                core = 4 * b + p
                sl = slice(CH * p, CH * (p + 1))
                wcat[core, :C, t * CH : (t + 1) * CH] = wt[sl].T
                wcat[core, C, t * CH : (t + 1) * CH] = bs[t][sl]
    return xg, xq, wcat.reshape(NCORES * (C + 1), 3 * CH).astype(np.float16)


def kernel(x, wq, bq, wk, bk, wv, bv, gamma):
    st = _get_state()
    jax = st["jax"]
    xg, xq, wcat_all = _host_pre(x, wq, bq, wk, bk, wv, bv, gamma)
    # x[b] goes to the group leader (core 0 / core 4) as one 256KB message;
    # the other cores get the persistent zero shards (no transfer).
    shards = list(st["xs_zero_shards"])
    shards[0] = jax.device_put(xq[:C], st["devices"][0])
    shards[4] = jax.device_put(xq[C:], st["devices"][4])
    xs_g = jax.make_array_from_single_device_arrays(
        (NCORES * C, N), st["csh"], shards
    )
    arrs = {"xs": xs_g, "wcat": wcat_all}
    outs = st["run"](*[arrs[n] for n in st["in_names"]], *st["zeros"])
    aug = jax.device_get(outs[0])                 # (128, 4096+32) int8
    sc = np.ascontiguousarray(aug[:, N:]).view(np.float32)   # (128, 8)
    # zero-copy (rows, block, col) view of the data columns; reshape on the
    # strided slice would silently copy the int8 first
    i8v = np.lib.stride_tricks.as_strided(
        aug, (B * C, NBLK, 512), (N + 4 * NBLK, 512, 1)
    )
    out = np.empty((B * C, N), np.float32)
    np.multiply(i8v, sc[:, :, None], out=out.reshape(B * C, NBLK, 512))
    out += xg                                     # exact f32 residual
    return out.reshape(B, C, 64, 64)


if __name__ == "__main__":
    rng = np.random.default_rng(0)
    x = rng.standard_normal((B, C, 64, 64), dtype=np.float32)
    wq, wk, wv = (
        rng.standard_normal((C, C), dtype=np.float32) / 8.0 for _ in range(3)
    )
    bq, bk, bv = (
        rng.standard_normal((C,), dtype=np.float32) * 0.01 for _ in range(3)
    )
    gamma = rng.random((1,), dtype=np.float32)
    out = kernel(x, wq, bq, wk, bk, wv, bv, gamma)
    print(out.shape, out.dtype)
